# revision 1
# baseline (speedup 1.0000x reference)
"""Trainium2 Bass kernel for ConvexDisplacementUpdate (B=4, L=4096, D=256).

new_coords = alpha * softmax(10 * qhat @ khat^T) @ coords + (1-alpha) * coords
q = l2norm(latents @ Wq^T), k = l2norm(latents @ Wk^T)  (row-wise l2norm)

Strategy (flash-attention style; the [L, L] score matrix never touches HBM):
  - 8 cores = (4 batches) x (2 query halves of 2048 rows). Host rolls each
    core's per-batch data so its own query rows are always columns 0:2048
    of the transposed latents -> one SPMD program, no per-core control flow.
  - Scores are computed transposed, S^T[m, l] = k_m . qhat_l, with k left
    UN-normalized; the per-m factor 10/||k_m|| is a per-partition scale
    folded into the exp() activation.
  - softmax without max-subtraction (|scores| <= 10, exp is safe in fp32).
  - numerator and denominator come from one PE matmul per tile with the
    ones-augmented coords [x, y, 1] as the stationary operand, accumulated
    over all 32 m-tiles in PSUM.
  - final alpha-blend + division happen on host (B*L*2 elements, trivial).
"""

import numpy as np

B, L, D = 4, 4096, 256
HALF = L // 2  # 2048 query rows per core
NCORES = 8
INV_TEMP = 10.0

_CACHE = {}


def build_module(reps=1, use_f32r=True, phases=3, loop_n=0, qk_bf16=True):
    """Build + compile the SPMD Bass module (one program, 8 cores)."""
    from contextlib import ExitStack

    import concourse.bacc as bacc
    import concourse.mybir as mybir
    import concourse.tile as tile
    from concourse.bass import ts
    from concourse.masks import make_identity

    dt = mybir.dt
    f32 = dt.float32
    AF = mybir.ActivationFunctionType
    ALU = mybir.AluOpType

    fr = dt.float32r if use_f32r else f32
    qk = dt.bfloat16 if qk_bf16 else fr

    def mm(ap):
        return ap

    nc = bacc.Bacc("TRN2", target_bir_lowering=False, debug=False,
                   num_devices=NCORES)

    latT = nc.dram_tensor("latT", [D, L], f32, kind="ExternalInput")
    wqT_d = nc.dram_tensor("wqT", [D, D], f32, kind="ExternalInput")
    wkT_d = nc.dram_tensor("wkT", [D, D], f32, kind="ExternalInput")
    caug_hi_d = nc.dram_tensor("caug_hi", [128, 3 * (L // 128)], dt.bfloat16,
                               kind="ExternalInput")
    caug_lo_d = nc.dram_tensor("caug_lo", [128, 3 * (L // 128)], dt.bfloat16,
                               kind="ExternalInput")
    pv_d = nc.dram_tensor("pv", [3, HALF], f32, kind="ExternalOutput")

    NLT = L // 128        # 32 m-tiles
    NQT = HALF // 128     # 16 q l-tiles
    NMB = L // 512        # 8 m-blocks
    NLB = HALF // 512     # 4 l-blocks

    with tile.TileContext(nc) as tc:
        loop = tc.For_i(0, loop_n, 1) if loop_n else None
        if loop is not None:
            loop.__enter__()
        for _rep in range(reps):
            with ExitStack() as ctx:
                persist = ctx.enter_context(tc.tile_pool(name="persist", bufs=1))

                # ---- load inputs (small weights first, lat chunks
                # interleaved across the two d-tiles so the first matmuls
                # can start after ~1MB) ----
                wq = [persist.tile([128, D], fr, tag=f"wq{i}", name=f"wq{i}") for i in range(2)]
                wk = [persist.tile([128, D], fr, tag=f"wk{i}", name=f"wk{i}") for i in range(2)]
                for i in range(2):
                    nc.sync.dma_start(out=wq[i], in_=wqT_d[i * 128:(i + 1) * 128, :].bitcast(fr))
                    nc.sync.dma_start(out=wk[i], in_=wkT_d[i * 128:(i + 1) * 128, :].bitcast(fr))
                caug_hi = persist.tile([128, 3 * NLT], dt.bfloat16, tag="caug_hi")
                caug_lo = persist.tile([128, 3 * NLT], dt.bfloat16, tag="caug_lo")
                nc.sync.dma_start(out=caug_hi, in_=caug_hi_d[:, :])
                nc.sync.dma_start(out=caug_lo, in_=caug_lo_d[:, :])
                ident = persist.tile([128, 128], f32, tag="ident")
                make_identity(nc, ident)
                ones = persist.tile([128, 1], f32, tag="ones")
                nc.vector.memset(ones, 1.0)

                lat = [persist.tile([128, L], fr, tag=f"lat{i}", name=f"lat{i}") for i in range(2)]
                chunks = [(0, 512), (512, 512), (1024, 1024), (2048, 1024),
                          (3072, 1024)]
                for off, size in chunks:
                    for i in range(2):
                        nc.sync.dma_start(
                            out=lat[i][:, off:off + size],
                            in_=latT[i * 128:(i + 1) * 128,
                                     off:off + size].bitcast(fr))

                qT = [persist.tile([128, HALF], qk, tag=f"qT{i}", name=f"qT{i}") for i in range(2)]
                kT = [persist.tile([128, L], qk, tag=f"kT{i}", name=f"kT{i}") for i in range(2)]
                q_all = persist.tile([128, NQT * D], f32, tag="q_all")
                ssq_q = persist.tile([128, NQT], f32, tag="ssq_q")
                inv_q = persist.tile([128, NQT], f32, tag="inv_q")
                inv_kT = persist.tile([128, NLT], f32, tag="inv_kT")

                with ExitStack() as p1:
                    big_ps = p1.enter_context(
                        tc.tile_pool(name="big_ps", bufs=3, space="PSUM"))
                    tp_ps = p1.enter_context(
                        tc.tile_pool(name="tp_ps", bufs=2, space="PSUM"))
                    kssq_ps = p1.enter_context(
                        tc.tile_pool(name="kssq_ps", bufs=1, space="PSUM"))
                    sm = p1.enter_context(tc.tile_pool(name="p1_small", bufs=4))
                    qh_pool = p1.enter_context(tc.tile_pool(name="qhat", bufs=3))
                    sq_pool = p1.enter_context(tc.tile_pool(name="k_sq", bufs=4))

                    # ---- phase 1q-A: raw q [l, e] + row sum-squares
                    # (ACT Square + accum_out straight from PSUM) ----
                    for lt in range(NQT):
                        qle = big_ps.tile([128, D], f32, tag="big", name="qle")
                        nc.tensor.matmul(qle, mm(lat[0][:, ts(lt, 128)]),
                                         mm(wq[0]), start=True, stop=False)
                        nc.tensor.matmul(qle, mm(lat[1][:, ts(lt, 128)]),
                                         mm(wq[1]), start=False, stop=True)
                        nc.vector.tensor_copy(out=q_all[:, ts(lt, D)], in_=qle)
                        junk = sm.tile([128, D], f32, tag="sqj")
                        nc.scalar.activation(junk, qle, AF.Square,
                                             accum_out=ssq_q[:, lt:lt + 1])
                    nrm_q = persist.tile([128, NQT], f32, tag="nrm_q")
                    nc.scalar.activation(nrm_q, ssq_q, AF.Sqrt)
                    nc.vector.reciprocal(inv_q, nrm_q)

                    # ---- phase 1q-B: normalize + transpose to [e, l] ----
                    for lt in range(NQT):
                        qhat = qh_pool.tile([128, D], f32, tag="qhat")
                        nc.vector.tensor_scalar_mul(qhat, q_all[:, ts(lt, D)],
                                                    inv_q[:, lt:lt + 1])
                        for et in range(2):
                            tp = tp_ps.tile([128, 128], f32, tag="tp")
                            nc.tensor.transpose(tp, qhat[:, ts(et, 128)], ident)
                            nc.vector.tensor_copy(out=qT[et][:, ts(lt, 128)], in_=tp)

                    # ---- phase 1k: kT_raw [e, m]; ssq via N=1 matmuls
                    # straight into the transposed [m-tile] layout ----
                    kssq = kssq_ps.tile([128, NLT], f32, tag="kssq")
                    for mb in range(NMB):
                        sqs = []
                        for et in range(2):
                            kp = big_ps.tile([128, 512], f32, tag="big", name="kp")
                            nc.tensor.matmul(kp, mm(wk[0][:, ts(et, 128)]),
                                             mm(lat[0][:, ts(mb, 512)]),
                                             start=True, stop=False)
                            nc.tensor.matmul(kp, mm(wk[1][:, ts(et, 128)]),
                                             mm(lat[1][:, ts(mb, 512)]),
                                             start=False, stop=True)
                            nc.vector.tensor_copy(out=kT[et][:, ts(mb, 512)],
                                                  in_=kp)
                            sq = sq_pool.tile([128, 512], f32, tag="ksq")
                            nc.vector.tensor_mul(sq, kT[et][:, ts(mb, 512)],
                                                 kT[et][:, ts(mb, 512)])
                            sqs.append(sq)
                        for j in range(4):
                            col = 4 * mb + j
                            for et in range(2):
                                nc.tensor.matmul(kssq[:, col:col + 1],
                                                 sqs[et][:, ts(j, 128)], ones,
                                                 start=(et == 0), stop=(et == 1))
                    # 10/||k_m||: 1/sqrt(ssq/100)
                    nrm_k = persist.tile([128, NLT], f32, tag="nrm_k")
                    nc.scalar.activation(nrm_k, kssq, AF.Sqrt,
                                         scale=1.0 / (INV_TEMP * INV_TEMP))
                    nc.vector.reciprocal(inv_kT, nrm_k)

                if phases < 3:
                    with tc.tile_pool(name="dbg", bufs=1) as dbg:
                        dtile = dbg.tile([3, HALF], f32, name="dtile")
                        nc.vector.tensor_copy(out=dtile, in_=kT[0][0:3, 0:HALF])
                        nc.sync.dma_start(out=pv_d[:, :], in_=dtile)
                    continue

                # ---- phase 2: scores^T -> exp -> [coords|1]^T @ P^T ----
                # software-pipelined: pv matmuls of tile t-1 are emitted
                # after the scores matmuls of tile t so PE never waits on
                # ACT's exp.
                with ExitStack() as p2:
                    sp_ps = p2.enter_context(
                        tc.tile_pool(name="sp_ps", bufs=3, space="PSUM"))
                    pv_ps = p2.enter_context(
                        tc.tile_pool(name="pv_ps", bufs=1, space="PSUM"))
                    p_pool = p2.enter_context(tc.tile_pool(name="p_sb", bufs=5))
                    pv_all = pv_ps.tile([128, 512], f32, tag="pv")

                    def emit_pv(t, ptiles):
                        for lb in range(NLB):
                            prhs = ptiles[lb // 2][:, ts(lb % 2, 512)]
                            nc.tensor.matmul(
                                pv_all[32 * lb:32 * lb + 3, :],
                                caug_hi[:, ts(t, 3)], prhs,
                                start=(t == 0), stop=False,
                                tile_position=(0, 32 * lb))
                            nc.tensor.matmul(
                                pv_all[32 * lb:32 * lb + 3, :],
                                caug_lo[:, ts(t, 3)], prhs,
                                start=False, stop=(t == NLT - 1),
                                tile_position=(0, 32 * lb))

                    prev = None
                    for t in range(NLT):
                        cur = []
                        for j in range(2):
                            sp = sp_ps.tile([128, 1024], f32, tag="sp")
                            for h in range(2):
                                lb = 2 * j + h
                                nc.tensor.matmul(sp[:, ts(h, 512)],
                                                 mm(kT[0][:, ts(t, 128)]),
                                                 mm(qT[0][:, ts(lb, 512)]),
                                                 start=True, stop=False)
                                nc.tensor.matmul(sp[:, ts(h, 512)],
                                                 mm(kT[1][:, ts(t, 128)]),
                                                 mm(qT[1][:, ts(lb, 512)]),
                                                 start=False, stop=True)
                            p = p_pool.tile([128, 1024], dt.bfloat16, tag="p")
                            nc.scalar.activation(p, sp, AF.Exp,
                                                 scale=inv_kT[:, t:t + 1])
                            cur.append(p)
                        if prev is not None:
                            emit_pv(t - 1, prev)
                        prev = cur
                    emit_pv(NLT - 1, prev)
                    out_sb = p2.enter_context(tc.tile_pool(name="out_sb", bufs=2))
                    for lb in range(NLB):
                        ot = out_sb.tile([3, 512], f32, tag="ot")
                        nc.vector.tensor_copy(out=ot,
                                              in_=pv_all[32 * lb:32 * lb + 3, :])
                        nc.sync.dma_start(out=pv_d[:, ts(lb, 512)], in_=ot)

        if loop is not None:
            loop.__exit__(None, None, None)
    nc.compile()
    return nc


def _get_module():
    if "nc" not in _CACHE:
        _CACHE["nc"] = build_module()
    return _CACHE["nc"]


def make_in_maps(latents, current_coords, Wq, Wk):
    """Per-core input dicts. Core c -> batch c//2, query half c%2 (rolled
    so own query rows are always columns 0:2048)."""
    latents = np.asarray(latents, np.float32)
    coords = np.asarray(current_coords, np.float32)
    wqT = np.ascontiguousarray(np.asarray(Wq, np.float32).T)
    wkT = np.ascontiguousarray(np.asarray(Wk, np.float32).T)
    in_maps = []
    for c in range(NCORES):
        b, h = divmod(c, 2)
        lat_b = np.roll(latents[b], -HALF * h, axis=0)
        coo_b = np.roll(coords[b], -HALF * h, axis=0)
        aug = np.concatenate([coo_b, np.ones((L, 1), np.float32)], axis=1)
        caug = np.ascontiguousarray(
            aug.reshape(L // 128, 128, 3).transpose(1, 0, 2).reshape(128, -1))
        import ml_dtypes
        hi = caug.astype(ml_dtypes.bfloat16)
        lo = (caug - hi.astype(np.float32)).astype(ml_dtypes.bfloat16)
        in_maps.append({
            "latT": np.ascontiguousarray(lat_b.T),
            "wqT": wqT,
            "wkT": wkT,
            "caug_hi": hi,
            "caug_lo": lo,
        })
    return in_maps


def postprocess(results, current_coords, alpha):
    """Assemble (new_coords, displacement) from per-core pv = [num_x; num_y; den]."""
    coords = np.asarray(current_coords, np.float32)
    new_coords = np.empty((B, L, 2), np.float32)
    for c in range(NCORES):
        b, h = divmod(c, 2)
        pv = results[c]["pv"]
        wc = (pv[0:2, :] / pv[2:3, :]).T  # [2048, 2] = (W @ coords) rows
        rows = slice(h * HALF, (h + 1) * HALF)
        new_coords[b, rows] = alpha * wc + (1.0 - alpha) * coords[b, rows]
    displacement = new_coords - coords
    return new_coords, displacement


def kernel(latents, current_coords, Wq, Wk, alpha_raw, layer_idx=None):
    from concourse.bass_utils import run_bass_kernel_spmd

    nc = _get_module()
    in_maps = make_in_maps(latents, current_coords, Wq, Wk)
    res = run_bass_kernel_spmd(nc, in_maps, list(range(NCORES)))
    alpha = np.float32(1.0 / (1.0 + np.exp(-np.float64(np.asarray(alpha_raw)))))
    return postprocess(res.results, current_coords, alpha)



# revision 2
# speedup vs baseline: 1.9006x; 1.9006x over previous
"""Trainium2 Bass kernel for ConvexDisplacementUpdate (B=4, L=4096, D=256).

new_coords = alpha * softmax(10 * qhat @ khat^T) @ coords + (1-alpha) * coords
q = l2norm(latents @ Wq^T), k = l2norm(latents @ Wk^T)  (row-wise l2norm)

v2 strategy (vs the 170us baseline):
  - inputs (latents, weights) shipped as fp8e4 (1MB DMA instead of 4MB);
    all projections and scores run as fp8 DoubleRow matmuls (K=256
    contraction in one MM at 0.5 cycles/row). Softmax normalization
    cancels the quantization noise (measured rel err ~7e-4 vs 2e-2 gate).
  - q projected directly into [e, l] layout (no PE transposes); ssq_q via
    ACT Square + GPSIMD partition_all_reduce (result lands broadcast on
    all partitions), applied by DVE during the fp8 conversion. k left
    un-normalized; 10/||k_m|| comes from ones-vector churn matmuls and is
    folded into the exp() activation scale (per-partition).
  - exp split across engines: ~60% on ACT, the rest via the exp2 bit-trick
    (DVE int32 convert of s*log2e*2^23 + bias, bitcast to fp32, convert to
    bf16 on GPSIMD/DVE). Softmax cancels the ~3% trick error.
  - PV (softmax numerator+denominator) via single bf16 caug (no hi/lo
    split), tile_position-packed into one PSUM bank, accumulated over all
    32 m-tiles.
  - phase-2 PSUM uses 3 rotating [128,1024] pool tiles so the per-tile WAR
    tracking stays precise and exp never stalls; scores are emitted one
    tile ahead of the PV block to keep the PE FIFO from head-blocking exp
    inputs. Engine programs are emitted in dataflow order (q-proj, k-side,
    q-norm chain) because engine queues are strict FIFO -- a stalled op
    head-blocks everything behind it.
  - final alpha-blend + division on host (B*L*2 elements, trivial).
"""

import numpy as np

B, L, D = 4, 4096, 256
HALF = L // 2  # 2048 query rows per core
NCORES = 8
INV_TEMP = 10.0

_CACHE = {}


def build_module(reps=1, use_f32r=True, phases=3, loop_n=0, qk_bf16=False,
                 trick_cols=768, pool_conv_cols=512):
    """Build + compile the SPMD Bass module (one program, 8 cores).

    trick_cols: number of columns (multiple of 256) of each 2048-wide P
    tile whose exp is computed via the exp2 bit-trick on DVE (int32
    convert) instead of ACT; pool_conv_cols of the bitcast->bf16 converts
    go to GPSIMD, the rest to DVE. 0 = all exp on ACT.
    """
    from contextlib import ExitStack

    import concourse.bacc as bacc
    import concourse.bass_isa as bass_isa
    import concourse.mybir as mybir
    import concourse.tile as tile
    from concourse.bass import ts

    from concourse.alu_op_type import AluOpType

    dt = mybir.dt
    f32 = dt.float32
    bf16 = dt.bfloat16
    fp8 = dt.float8e4
    i32 = dt.int32
    AF = mybir.ActivationFunctionType
    PM = mybir.MatmulPerfMode

    # exp2 bit-trick constants: exp(s) ~= bitcast_f32(round(2^23*(s*log2e
    # + 127 - C))), max rel err ~3% -- softmax cancels most of it.
    LOG2E_SCALED = 1.4426950408889634 * (1 << 23)
    EXP_BIAS = (127.0 - 0.0430) * (1 << 23)

    NLT = L // 128        # 32 m-tiles
    NMB = L // 512        # 8 m 512-blocks
    NQB = HALF // 1024    # 2 q 1024-blocks per e-tile

    nc = bacc.Bacc("TRN2", target_bir_lowering=False, debug=False,
                   num_devices=NCORES)

    lat_d = nc.dram_tensor("lat8", [128, 2, L], fp8, kind="ExternalInput")
    wq_d = nc.dram_tensor("wq8", [128, 2, D], fp8, kind="ExternalInput")
    wk_d = nc.dram_tensor("wk8", [128, 2, D], fp8, kind="ExternalInput")
    caug_d = nc.dram_tensor("caug", [128, 3 * NLT], bf16, kind="ExternalInput")
    pv_d = nc.dram_tensor("pv", [3, HALF], f32, kind="ExternalOutput")

    with tile.TileContext(nc) as tc:
        loop = tc.For_i(0, loop_n, 1) if loop_n else None
        if loop is not None:
            loop.__enter__()
        for _rep in range(reps):
            with ExitStack() as ctx:
                persist = ctx.enter_context(tc.tile_pool(name="persist", bufs=1))

                # ---- input DMA: weights + caug on the ACT hwdge queue,
                # lat chunks on the SP queue (parallel issue) ----
                w8q = persist.tile([128, 2, D], fp8, tag="w8q")
                w8k = persist.tile([128, 2, D], fp8, tag="w8k")
                nc.scalar.dma_start(out=w8q, in_=wq_d[:, :, :])
                nc.scalar.dma_start(out=w8k, in_=wk_d[:, :, :])
                caug = persist.tile([128, 3 * NLT], bf16, tag="caug")
                nc.scalar.dma_start(out=caug, in_=caug_d[:, :])
                ones = persist.tile([128, 1], bf16, tag="ones")
                nc.vector.memset(ones, 1.0)

                lat8 = persist.tile([128, 2, L], fp8, tag="lat8")
                for off, size in [(0, 1024), (1024, 1024), (2048, 1024),
                                  (3072, 1024)]:
                    nc.sync.dma_start(out=lat8[:, :, off:off + size],
                                      in_=lat_d[:, :, off:off + size])

                # persistent compute tiles
                qh8 = persist.tile([128, 2, HALF], fp8, tag="qh8")
                k8 = persist.tile([128, 2, L], fp8, tag="k8")
                inv_k = persist.tile([128, NLT], f32, tag="inv_k")
                ktrick = persist.tile([128, NLT], f32, tag="ktrick")
                inv_q = persist.tile([128, HALF], f32, tag="inv_q")
                qraw = [persist.tile([128, HALF], f32, tag=f"qraw{i}", name=f"qraw{i}")
                        for i in range(2)]

                with ExitStack() as p1:
                    proj_ps = p1.enter_context(
                        tc.tile_pool(name="proj_ps", bufs=3, space="PSUM"))
                    ssqk_ps = p1.enter_context(
                        tc.tile_pool(name="ssqk_ps", bufs=1, space="PSUM"))
                    sqk_pool = p1.enter_context(tc.tile_pool(name="sqk", bufs=4))
                    sqq_pool = p1.enter_context(tc.tile_pool(name="sqq", bufs=1))

                    ssqk = ssqk_ps.tile([128, NLT], f32, tag="ssqk")
                    sqq = sqq_pool.tile([128, 2 * HALF], f32, tag="sqq")

                    # ---- q projection: qT_raw [e, l] (no transposes) ----
                    # squares read PSUM in parallel with the qraw copy; the
                    # whole norm chain is chunked so it pipelines and overlaps
                    # the k projection below. NB: GPSIMD cannot touch PSUM,
                    # so all PSUM egress is on DVE/ACT.
                    for et in range(2):
                        for qb in range(NQB):
                            qp = proj_ps.tile([128, 1024], f32, tag="kp",
                                              name=f"qp{et}_{qb}")
                            for h in range(2):
                                sl = ts(2 * qb + h, 512)
                                nc.tensor.matmul(qp[:, ts(h, 512)],
                                                 w8q[:, :, ts(et, 128)],
                                                 lat8[:, :, sl],
                                                 start=True, stop=True,
                                                 perf_mode=PM.DoubleRow)
                            nc.vector.tensor_copy(
                                out=qraw[et][:, ts(qb, 1024)], in_=qp)
                            nc.scalar.activation(
                                sqq[:, et * HALF + qb * 1024:
                                    et * HALF + (qb + 1) * 1024],
                                qp, AF.Square)
                    # ---- k projection: kT_raw [e, m] -> fp8 k8 + ssq ----
                    for mbb in range(4):  # 1024-wide m blocks
                        sqs = []
                        for et in range(2):
                            kp = proj_ps.tile([128, 1024], f32, tag="kp",
                                              name=f"kp{mbb}_{et}")
                            for h in range(2):
                                nc.tensor.matmul(kp[:, ts(h, 512)],
                                                 w8k[:, :, ts(et, 128)],
                                                 lat8[:, :, ts(2 * mbb + h, 512)],
                                                 start=True, stop=True,
                                                 perf_mode=PM.DoubleRow)
                            nc.vector.tensor_copy(
                                out=k8[:, et, ts(mbb, 1024)], in_=kp)
                            # squares for ||k_m||^2 (bf16 out for churn MMs)
                            sq = sqk_pool.tile([128, 1024], bf16, tag="ksq")
                            nc.scalar.activation(sq, kp, AF.Square)
                            sqs.append(sq)
                        for j in range(8):
                            col = 8 * mbb + j
                            nc.tensor.matmul(ssqk[:, col:col + 1],
                                             sqs[0][:, ts(j, 128)], ones,
                                             start=True, stop=False)
                            nc.tensor.matmul(ssqk[:, col:col + 1],
                                             sqs[1][:, ts(j, 128)], ones,
                                             start=False, stop=True)
                        # inv_k = 10/||k_m|| = 1/sqrt(ssq/100); chunked so the
                        # first exps don't wait on the whole k projection
                        if mbb in (0, 3):
                            sl = slice(0, 8) if mbb == 0 else slice(8, NLT)
                            nrm_k = sqk_pool.tile([128, NLT], f32, tag="nrm_k",
                                                  name=f"nrm_k{mbb}")
                            nc.scalar.activation(nrm_k[:, sl], ssqk[:, sl],
                                                 AF.Sqrt,
                                                 scale=1.0 / (INV_TEMP * INV_TEMP))
                            nc.vector.reciprocal(inv_k[:, sl], nrm_k[:, sl])
                            if trick_cols:
                                nc.vector.tensor_scalar_mul(
                                    ktrick[:, sl], inv_k[:, sl], LOG2E_SCALED)

                    # ssq_q -> inv_q, chunked 4x512 for pipelining. GPSIMD
                    # runs ONLY partition_all_reduce: mixing op types on the
                    # Q7s forces a library reload per switch (us-scale on HW).
                    sqsum = sqq_pool.tile([128, HALF], f32, tag="sqsum")
                    for cc in range(4):
                        sl = ts(cc, 512)
                        nc.vector.tensor_add(sqsum[:, sl], sqq[:, sl],
                                             sqq[:, HALF + cc * 512:
                                                 HALF + (cc + 1) * 512])
                        nc.gpsimd.partition_all_reduce(
                            sqsum[:, sl], sqsum[:, sl], channels=128,
                            reduce_op=bass_isa.ReduceOp.add)
                        nc.scalar.activation(sqsum[:, sl], sqsum[:, sl],
                                             AF.Sqrt)
                        nc.vector.reciprocal(inv_q[:, sl], sqsum[:, sl])
                        for et in range(2):
                            nc.vector.tensor_mul(qh8[:, et, sl],
                                                 qraw[et][:, sl],
                                                 inv_q[:, sl])

                if phases < 3:
                    with tc.tile_pool(name="dbg", bufs=1) as dbg:
                        dtile = dbg.tile([3, HALF], f32, name="dtile")
                        nc.vector.tensor_copy(out=dtile, in_=qraw[0][0:3, :])
                        nc.sync.dma_start(out=pv_d[:, :], in_=dtile)
                    continue

                # ---- phase 2: scores^T -> exp -> PV accumulation ----
                with ExitStack() as p2:
                    sp_ps = p2.enter_context(
                        tc.tile_pool(name="sp_ps", bufs=3, space="PSUM"))
                    pv_ps = p2.enter_context(
                        tc.tile_pool(name="pv_ps", bufs=1, space="PSUM"))
                    p_pool = p2.enter_context(tc.tile_pool(name="p_sb", bufs=4))
                    i32_pool = p2.enter_context(
                        tc.tile_pool(name="i32_sb", bufs=3))
                    pv_all = pv_ps.tile([128, 512], f32, tag="pv")

                    def emit_scores(t):
                        # two pool tiles per m-tile; distinct tile objects
                        # keep the WAR tracking per-tile (precise), so these
                        # MMs only wait on exp(t-2) -- lots of slack.
                        sps = []
                        for half in range(2):
                            sp = sp_ps.tile([128, 1024], f32, tag="sp",
                                            name=f"sp{t}_{half}")
                            for h in range(2):
                                nc.tensor.matmul(sp[:, ts(h, 512)],
                                                 k8[:, :, ts(t, 128)],
                                                 qh8[:, :, ts(2 * half + h, 512)],
                                                 start=True, stop=True,
                                                 perf_mode=PM.DoubleRow)
                            sps.append(sp)
                        return sps

                    def emit_exp(t, sps, p):
                        spA, spB = sps
                        nc.scalar.activation(p[:, 0:1024], spA, AF.Exp,
                                             scale=inv_k[:, t:t + 1])
                        na = 1024 - trick_cols  # ACT cols within spB
                        if na:
                            nc.scalar.activation(p[:, 1024:1024 + na],
                                                 spB[:, 0:na], AF.Exp,
                                                 scale=inv_k[:, t:t + 1])
                        if trick_cols:
                            it = i32_pool.tile([128, trick_cols], i32,
                                               tag="i32", name=f"i32_{t}")
                            nc.vector.tensor_scalar(
                                out=it, in0=spB[:, na:1024],
                                scalar1=ktrick[:, t:t + 1], scalar2=EXP_BIAS,
                                op0=AluOpType.mult, op1=AluOpType.add)
                            pc = min(pool_conv_cols, trick_cols)
                            if pc:
                                nc.gpsimd.tensor_copy(
                                    out=p[:, 1024 + na:1024 + na + pc],
                                    in_=it[:, 0:pc].bitcast(f32))
                            if pc < trick_cols:
                                nc.vector.tensor_copy(
                                    out=p[:, 1024 + na + pc:2048],
                                    in_=it[:, pc:trick_cols].bitcast(f32))

                    def emit_pv(t, p):
                        for lb in range(4):
                            nc.tensor.matmul(
                                pv_all[32 * lb:32 * lb + 3, :],
                                caug[:, 3 * t:3 * t + 3], p[:, ts(lb, 512)],
                                start=(t == 0), stop=(t == NLT - 1),
                                tile_position=(0, 32 * lb))

                    # scores are emitted one tile ahead of the pv block so
                    # the PE queue never holds exp(t)'s inputs hostage behind
                    # pv (which waits on exp output).
                    ptiles = {}
                    sps = {0: emit_scores(0)}
                    for t in range(NLT):
                        ptiles[t] = p_pool.tile([128, HALF], bf16, tag="p",
                                                name=f"p{t}")
                        emit_exp(t, sps.pop(t), ptiles[t])
                        if t + 1 < NLT:
                            sps[t + 1] = emit_scores(t + 1)
                        if t - 2 >= 0:
                            emit_pv(t - 2, ptiles.pop(t - 2))
                    emit_pv(NLT - 2, ptiles.pop(NLT - 2))
                    emit_pv(NLT - 1, ptiles.pop(NLT - 1))

                    out_sb = p2.enter_context(tc.tile_pool(name="out_sb", bufs=4))
                    for lb in range(4):
                        ot = out_sb.tile([3, 512], f32, tag="ot")
                        # split the drain across DVE/ACT and both hwdge
                        # queues so the tail isn't serialized on one engine
                        if lb % 2 == 0:
                            nc.vector.tensor_copy(out=ot,
                                                  in_=pv_all[32 * lb:32 * lb + 3, :])
                            nc.sync.dma_start(out=pv_d[:, ts(lb, 512)], in_=ot)
                        else:
                            nc.scalar.activation(ot,
                                                 pv_all[32 * lb:32 * lb + 3, :],
                                                 AF.Copy)
                            nc.scalar.dma_start(out=pv_d[:, ts(lb, 512)], in_=ot)

        if loop is not None:
            loop.__exit__(None, None, None)
    nc.compile()
    return nc


def _get_module():
    if "nc" not in _CACHE:
        _CACHE["nc"] = build_module()
    return _CACHE["nc"]


def make_in_maps(latents, current_coords, Wq, Wk):
    """Per-core input dicts. Core c -> batch c//2, query half c%2 (rolled
    so own query rows are always columns 0:2048)."""
    import ml_dtypes
    fp8 = ml_dtypes.float8_e4m3fn
    latents = np.asarray(latents, np.float32)
    coords = np.asarray(current_coords, np.float32)

    def dhalves(mat_T):  # [256, N] -> [128, 2, N] (partition, d-half, col)
        return np.ascontiguousarray(
            mat_T.reshape(2, 128, -1).transpose(1, 0, 2)).astype(fp8)

    wq8 = dhalves(np.ascontiguousarray(np.asarray(Wq, np.float32).T))
    wk8 = dhalves(np.ascontiguousarray(np.asarray(Wk, np.float32).T))
    in_maps = []
    for c in range(NCORES):
        b, h = divmod(c, 2)
        lat_b = np.roll(latents[b], -HALF * h, axis=0)
        coo_b = np.roll(coords[b], -HALF * h, axis=0)
        aug = np.concatenate([coo_b, np.ones((L, 1), np.float32)], axis=1)
        caug = np.ascontiguousarray(
            aug.reshape(L // 128, 128, 3).transpose(1, 0, 2).reshape(128, -1))
        in_maps.append({
            "lat8": dhalves(np.ascontiguousarray(lat_b.T)),
            "wq8": wq8,
            "wk8": wk8,
            "caug": caug.astype(ml_dtypes.bfloat16),
        })
    return in_maps


def postprocess(results, current_coords, alpha):
    """Assemble (new_coords, displacement) from per-core pv = [num_x; num_y; den]."""
    coords = np.asarray(current_coords, np.float32)
    new_coords = np.empty((B, L, 2), np.float32)
    for c in range(NCORES):
        b, h = divmod(c, 2)
        pv = results[c]["pv"]
        wc = (pv[0:2, :] / pv[2:3, :]).T  # [2048, 2] = (W @ coords) rows
        rows = slice(h * HALF, (h + 1) * HALF)
        new_coords[b, rows] = alpha * wc + (1.0 - alpha) * coords[b, rows]
    displacement = new_coords - coords
    return new_coords, displacement


def kernel(latents, current_coords, Wq, Wk, alpha_raw, layer_idx=None):
    from concourse.bass_utils import run_bass_kernel_spmd

    nc = _get_module()
    in_maps = make_in_maps(latents, current_coords, Wq, Wk)
    res = run_bass_kernel_spmd(nc, in_maps, list(range(NCORES)))
    alpha = np.float32(1.0 / (1.0 + np.exp(-np.float64(np.asarray(alpha_raw)))))
    return postprocess(res.results, current_coords, alpha)


# revision 3
# speedup vs baseline: 2.6102x; 1.3733x over previous
"""Trainium2 Bass kernel for ConvexDisplacementUpdate (B=4, L=4096, D=256).

new_coords = alpha * softmax(10 * qhat @ khat^T) @ coords + (1-alpha) * coords
q = l2norm(latents @ Wq^T), k = l2norm(latents @ Wk^T)  (row-wise l2norm)

v2 strategy (vs the 170us baseline):
  - inputs (latents, weights) shipped as fp8e4 (1MB DMA instead of 4MB);
    all projections and scores run as fp8 DoubleRow matmuls (K=256
    contraction in one MM at 0.5 cycles/row). Softmax normalization
    cancels the quantization noise (measured rel err ~7e-4 vs 2e-2 gate).
  - q projected directly into [e, l] layout (no PE transposes); ssq_q via
    ACT Square + GPSIMD partition_all_reduce (result lands broadcast on
    all partitions), applied by DVE during the fp8 conversion. k left
    un-normalized; 10/||k_m|| comes from ones-vector churn matmuls and is
    folded into the exp() activation scale (per-partition).
  - exp split across engines: ~60% on ACT, the rest via the exp2 bit-trick
    (DVE int32 convert of s*log2e*2^23 + bias, bitcast to fp32, convert to
    bf16 on GPSIMD/DVE). Softmax cancels the ~3% trick error.
  - PV (softmax numerator+denominator) via single bf16 caug (no hi/lo
    split), tile_position-packed into one PSUM bank, accumulated over all
    32 m-tiles.
  - phase-2 PSUM uses 3 rotating [128,1024] pool tiles so the per-tile WAR
    tracking stays precise and exp never stalls; scores are emitted one
    tile ahead of the PV block to keep the PE FIFO from head-blocking exp
    inputs. Engine programs are emitted in dataflow order (q-proj, k-side,
    q-norm chain) because engine queues are strict FIFO -- a stalled op
    head-blocks everything behind it.
  - final alpha-blend + division on host (B*L*2 elements, trivial).
"""

import numpy as np

B, L, D = 4, 4096, 256
HALF = L // 2  # 2048 query rows per core
NCORES = 8
INV_TEMP = 10.0

_CACHE = {}


def build_module(reps=1, use_f32r=True, phases=3, loop_n=0, qk_bf16=False,
                 trick_cols=768, pool_conv_cols=512):
    """Build + compile the SPMD Bass module (one program, 8 cores).

    trick_cols: number of columns (multiple of 256) of each 2048-wide P
    tile whose exp is computed via the exp2 bit-trick on DVE (int32
    convert) instead of ACT; pool_conv_cols of the bitcast->bf16 converts
    go to GPSIMD, the rest to DVE. 0 = all exp on ACT.
    """
    from contextlib import ExitStack

    import concourse.bacc as bacc
    import concourse.bass_isa as bass_isa
    import concourse.mybir as mybir
    import concourse.tile as tile
    from concourse.bass import ts

    from concourse.alu_op_type import AluOpType

    dt = mybir.dt
    f32 = dt.float32
    bf16 = dt.bfloat16
    fp8 = dt.float8e4
    i32 = dt.int32
    AF = mybir.ActivationFunctionType
    PM = mybir.MatmulPerfMode

    # exp2 bit-trick constants: exp(s) ~= bitcast_f32(round(2^23*(s*log2e
    # + 127 - C))), max rel err ~3% -- softmax cancels most of it.
    LOG2E_SCALED = 1.4426950408889634 * (1 << 23)
    EXP_BIAS = (127.0 - 0.0430) * (1 << 23)

    NLT = L // 128        # 32 m-tiles
    NMB = L // 512        # 8 m 512-blocks
    NQB = HALF // 1024    # 2 q 1024-blocks per e-tile

    nc = bacc.Bacc("TRN2", target_bir_lowering=False, debug=False,
                   num_devices=NCORES)

    lat_d = nc.dram_tensor("lat8", [128, 2, L], fp8, kind="ExternalInput")
    wq_d = nc.dram_tensor("wq8", [128, 2, D], fp8, kind="ExternalInput")
    wk_d = nc.dram_tensor("wk8", [128, 2, D], fp8, kind="ExternalInput")
    caug_d = nc.dram_tensor("caug", [128, 3 * NLT], bf16, kind="ExternalInput")
    pv_d = nc.dram_tensor("pv", [3, HALF], f32, kind="ExternalOutput")

    with tile.TileContext(nc) as tc:
        loop = tc.For_i(0, loop_n, 1) if loop_n else None
        if loop is not None:
            loop.__enter__()
        for _rep in range(reps):
            with ExitStack() as ctx:
                persist = ctx.enter_context(tc.tile_pool(name="persist", bufs=1))

                # ---- input DMA: weights + caug on the ACT hwdge queue,
                # lat chunks on the SP queue (parallel issue) ----
                w8q = persist.tile([128, 2, D], fp8, tag="w8q")
                w8k = persist.tile([128, 2, D], fp8, tag="w8k")
                nc.scalar.dma_start(out=w8q, in_=wq_d[:, :, :])
                nc.scalar.dma_start(out=w8k, in_=wk_d[:, :, :])
                caug = persist.tile([128, 3 * NLT], bf16, tag="caug")
                nc.scalar.dma_start(out=caug, in_=caug_d[:, :])
                ones = persist.tile([128, 1], bf16, tag="ones")
                nc.vector.memset(ones, 1.0)

                lat8 = persist.tile([128, 2, L], fp8, tag="lat8")
                for off, size in [(0, 1024), (1024, 1024), (2048, 1024),
                                  (3072, 1024)]:
                    nc.sync.dma_start(out=lat8[:, :, off:off + size],
                                      in_=lat_d[:, :, off:off + size])

                # persistent compute tiles
                qh8 = persist.tile([128, 2, HALF], fp8, tag="qh8")
                k8 = persist.tile([128, 2, L], fp8, tag="k8")
                inv_k = persist.tile([128, NLT], f32, tag="inv_k")
                ktrick = persist.tile([128, NLT], f32, tag="ktrick")
                inv_q = persist.tile([128, HALF], f32, tag="inv_q")
                qraw = [persist.tile([128, HALF], f32, tag=f"qraw{i}", name=f"qraw{i}")
                        for i in range(2)]

                with ExitStack() as p1:
                    proj_ps = p1.enter_context(
                        tc.tile_pool(name="proj_ps", bufs=3, space="PSUM"))
                    ssqk_ps = p1.enter_context(
                        tc.tile_pool(name="ssqk_ps", bufs=1, space="PSUM"))
                    sqk_pool = p1.enter_context(tc.tile_pool(name="sqk", bufs=4))
                    sqq_pool = p1.enter_context(tc.tile_pool(name="sqq", bufs=1))

                    ssqk = ssqk_ps.tile([128, NLT], f32, tag="ssqk")
                    sqq = sqq_pool.tile([128, 2 * HALF], f32, tag="sqq")

                    # ---- q projection: qT_raw [e, l] (no transposes) ----
                    # squares read PSUM in parallel with the qraw copy; the
                    # whole norm chain is chunked so it pipelines and overlaps
                    # the k projection below. NB: GPSIMD cannot touch PSUM,
                    # so all PSUM egress is on DVE/ACT.
                    for et in range(2):
                        for qb in range(NQB):
                            qp = proj_ps.tile([128, 1024], f32, tag="kp",
                                              name=f"qp{et}_{qb}")
                            for h in range(2):
                                sl = ts(2 * qb + h, 512)
                                nc.tensor.matmul(qp[:, ts(h, 512)],
                                                 w8q[:, :, ts(et, 128)],
                                                 lat8[:, :, sl],
                                                 start=True, stop=True,
                                                 perf_mode=PM.DoubleRow)
                            nc.vector.tensor_copy(
                                out=qraw[et][:, ts(qb, 1024)], in_=qp)
                            nc.scalar.activation(
                                sqq[:, et * HALF + qb * 1024:
                                    et * HALF + (qb + 1) * 1024],
                                qp, AF.Square)
                    # ---- k projection: kT_raw [e, m] -> fp8 k8 + ssq ----
                    for mbb in range(4):  # 1024-wide m blocks
                        sqs = []
                        for et in range(2):
                            kp = proj_ps.tile([128, 1024], f32, tag="kp",
                                              name=f"kp{mbb}_{et}")
                            for h in range(2):
                                nc.tensor.matmul(kp[:, ts(h, 512)],
                                                 w8k[:, :, ts(et, 128)],
                                                 lat8[:, :, ts(2 * mbb + h, 512)],
                                                 start=True, stop=True,
                                                 perf_mode=PM.DoubleRow)
                            nc.vector.tensor_copy(
                                out=k8[:, et, ts(mbb, 1024)], in_=kp)
                            # squares for ||k_m||^2 (bf16 out for churn MMs)
                            sq = sqk_pool.tile([128, 1024], bf16, tag="ksq")
                            nc.scalar.activation(sq, kp, AF.Square)
                            sqs.append(sq)
                        for j in range(8):
                            col = 8 * mbb + j
                            nc.tensor.matmul(ssqk[:, col:col + 1],
                                             sqs[0][:, ts(j, 128)], ones,
                                             start=True, stop=False)
                            nc.tensor.matmul(ssqk[:, col:col + 1],
                                             sqs[1][:, ts(j, 128)], ones,
                                             start=False, stop=True)
                        # inv_k = 10/||k_m|| = 1/sqrt(ssq/100); chunked so the
                        # first exps don't wait on the whole k projection
                        if mbb == 3:
                            sl = slice(0, NLT)
                            nrm_k = sqk_pool.tile([128, NLT], f32, tag="nrm_k",
                                                  name=f"nrm_k{mbb}")
                            nc.scalar.activation(nrm_k[:, sl], ssqk[:, sl],
                                                 AF.Sqrt,
                                                 scale=1.0 / (INV_TEMP * INV_TEMP))
                            nc.vector.reciprocal(inv_k[:, sl], nrm_k[:, sl])
                            if trick_cols:
                                nc.vector.tensor_scalar_mul(
                                    ktrick[:, sl], inv_k[:, sl], LOG2E_SCALED)

                    # ssq_q -> inv_q, chunked 4x512 for pipelining. GPSIMD
                    # runs ONLY partition_all_reduce: mixing op types on the
                    # Q7s forces a library reload per switch (us-scale on HW).
                    sqsum = sqq_pool.tile([128, HALF], f32, tag="sqsum")
                    for cc in range(2):
                        sl = ts(cc, 1024)
                        nc.vector.tensor_add(sqsum[:, sl], sqq[:, sl],
                                             sqq[:, HALF + cc * 1024:
                                                 HALF + (cc + 1) * 1024])
                        nc.gpsimd.partition_all_reduce(
                            sqsum[:, sl], sqsum[:, sl], channels=128,
                            reduce_op=bass_isa.ReduceOp.add)
                        nc.scalar.activation(sqsum[:, sl], sqsum[:, sl],
                                             AF.Sqrt)
                        nc.vector.reciprocal(inv_q[:, sl], sqsum[:, sl])
                        for et in range(2):
                            nc.vector.tensor_mul(qh8[:, et, sl],
                                                 qraw[et][:, sl],
                                                 inv_q[:, sl])

                if phases < 3:
                    with tc.tile_pool(name="dbg", bufs=1) as dbg:
                        dtile = dbg.tile([3, HALF], f32, name="dtile")
                        nc.vector.tensor_copy(out=dtile, in_=qraw[0][0:3, :])
                        nc.sync.dma_start(out=pv_d[:, :], in_=dtile)
                    continue

                # ---- phase 2: scores^T -> exp -> PV accumulation ----
                with ExitStack() as p2:
                    sp_ps = p2.enter_context(
                        tc.tile_pool(name="sp_ps", bufs=3, space="PSUM"))
                    pv_ps = p2.enter_context(
                        tc.tile_pool(name="pv_ps", bufs=1, space="PSUM"))
                    p_pool = p2.enter_context(tc.tile_pool(name="p_sb", bufs=4))
                    i32_pool = p2.enter_context(
                        tc.tile_pool(name="i32_sb", bufs=3))
                    pv_all = pv_ps.tile([128, 512], f32, tag="pv")

                    def emit_scores(t):
                        # two pool tiles per m-tile; distinct tile objects
                        # keep the WAR tracking per-tile (precise), so these
                        # MMs only wait on exp(t-2) -- lots of slack.
                        sps = []
                        for half in range(2):
                            sp = sp_ps.tile([128, 1024], f32, tag="sp",
                                            name=f"sp{t}_{half}")
                            for h in range(2):
                                nc.tensor.matmul(sp[:, ts(h, 512)],
                                                 k8[:, :, ts(t, 128)],
                                                 qh8[:, :, ts(2 * half + h, 512)],
                                                 start=True, stop=True,
                                                 perf_mode=PM.DoubleRow)
                            sps.append(sp)
                        return sps

                    def emit_exp(t, sps, p):
                        spA, spB = sps
                        nc.scalar.activation(p[:, 0:1024], spA, AF.Exp,
                                             scale=inv_k[:, t:t + 1])
                        na = 1024 - trick_cols  # ACT cols within spB
                        if na:
                            nc.scalar.activation(p[:, 1024:1024 + na],
                                                 spB[:, 0:na], AF.Exp,
                                                 scale=inv_k[:, t:t + 1])
                        if trick_cols:
                            it = i32_pool.tile([128, trick_cols], i32,
                                               tag="i32", name=f"i32_{t}")
                            nc.vector.tensor_scalar(
                                out=it, in0=spB[:, na:1024],
                                scalar1=ktrick[:, t:t + 1], scalar2=EXP_BIAS,
                                op0=AluOpType.mult, op1=AluOpType.add)
                            pc = min(pool_conv_cols, trick_cols)
                            if pc:
                                nc.gpsimd.tensor_copy(
                                    out=p[:, 1024 + na:1024 + na + pc],
                                    in_=it[:, 0:pc].bitcast(f32))
                            if pc < trick_cols:
                                nc.vector.tensor_copy(
                                    out=p[:, 1024 + na + pc:2048],
                                    in_=it[:, pc:trick_cols].bitcast(f32))

                    def emit_pv(t, p):
                        for lb in range(4):
                            nc.tensor.matmul(
                                pv_all[32 * lb:32 * lb + 3, :],
                                caug[:, 3 * t:3 * t + 3], p[:, ts(lb, 512)],
                                start=(t == 0), stop=(t == NLT - 1),
                                tile_position=(0, 32 * lb))

                    # scores are emitted one tile ahead of the pv block so
                    # the PE queue never holds exp(t)'s inputs hostage behind
                    # pv (which waits on exp output).
                    ptiles = {}
                    sps = {0: emit_scores(0)}
                    for t in range(NLT):
                        ptiles[t] = p_pool.tile([128, HALF], bf16, tag="p",
                                                name=f"p{t}")
                        emit_exp(t, sps.pop(t), ptiles[t])
                        if t + 1 < NLT:
                            sps[t + 1] = emit_scores(t + 1)
                        if t - 2 >= 0:
                            emit_pv(t - 2, ptiles.pop(t - 2))
                    emit_pv(NLT - 2, ptiles.pop(NLT - 2))
                    emit_pv(NLT - 1, ptiles.pop(NLT - 1))

                    out_sb = p2.enter_context(tc.tile_pool(name="out_sb", bufs=4))
                    for lb in range(4):
                        ot = out_sb.tile([3, 512], f32, tag="ot")
                        # split the drain across DVE/ACT and both hwdge
                        # queues so the tail isn't serialized on one engine
                        if lb % 2 == 0:
                            nc.vector.tensor_copy(out=ot,
                                                  in_=pv_all[32 * lb:32 * lb + 3, :])
                            nc.sync.dma_start(out=pv_d[:, ts(lb, 512)], in_=ot)
                        else:
                            nc.scalar.activation(ot,
                                                 pv_all[32 * lb:32 * lb + 3, :],
                                                 AF.Copy)
                            nc.scalar.dma_start(out=pv_d[:, ts(lb, 512)], in_=ot)

        if loop is not None:
            loop.__exit__(None, None, None)
    nc.compile()
    return nc


def _get_module():
    if "nc" not in _CACHE:
        _CACHE["nc"] = build_module()
    return _CACHE["nc"]


def make_in_maps(latents, current_coords, Wq, Wk):
    """Per-core input dicts. Core c -> batch c//2, query half c%2 (rolled
    so own query rows are always columns 0:2048)."""
    import ml_dtypes
    fp8 = ml_dtypes.float8_e4m3fn
    latents = np.asarray(latents, np.float32)
    coords = np.asarray(current_coords, np.float32)

    def dhalves(mat_T):  # [256, N] -> [128, 2, N] (partition, d-half, col)
        return np.ascontiguousarray(
            mat_T.reshape(2, 128, -1).transpose(1, 0, 2)).astype(fp8)

    wq8 = dhalves(np.ascontiguousarray(np.asarray(Wq, np.float32).T))
    wk8 = dhalves(np.ascontiguousarray(np.asarray(Wk, np.float32).T))
    in_maps = []
    for c in range(NCORES):
        b, h = divmod(c, 2)
        lat_b = np.roll(latents[b], -HALF * h, axis=0)
        coo_b = np.roll(coords[b], -HALF * h, axis=0)
        aug = np.concatenate([coo_b, np.ones((L, 1), np.float32)], axis=1)
        caug = np.ascontiguousarray(
            aug.reshape(L // 128, 128, 3).transpose(1, 0, 2).reshape(128, -1))
        in_maps.append({
            "lat8": dhalves(np.ascontiguousarray(lat_b.T)),
            "wq8": wq8,
            "wk8": wk8,
            "caug": caug.astype(ml_dtypes.bfloat16),
        })
    return in_maps


def postprocess(results, current_coords, alpha):
    """Assemble (new_coords, displacement) from per-core pv = [num_x; num_y; den]."""
    coords = np.asarray(current_coords, np.float32)
    new_coords = np.empty((B, L, 2), np.float32)
    for c in range(NCORES):
        b, h = divmod(c, 2)
        pv = results[c]["pv"]
        wc = (pv[0:2, :] / pv[2:3, :]).T  # [2048, 2] = (W @ coords) rows
        rows = slice(h * HALF, (h + 1) * HALF)
        new_coords[b, rows] = alpha * wc + (1.0 - alpha) * coords[b, rows]
    displacement = new_coords - coords
    return new_coords, displacement


def kernel(latents, current_coords, Wq, Wk, alpha_raw, layer_idx=None):
    from concourse.bass_utils import run_bass_kernel_spmd

    nc = _get_module()
    in_maps = make_in_maps(latents, current_coords, Wq, Wk)
    res = run_bass_kernel_spmd(nc, in_maps, list(range(NCORES)))
    alpha = np.float32(1.0 / (1.0 + np.exp(-np.float64(np.asarray(alpha_raw)))))
    return postprocess(res.results, current_coords, alpha)
